# revision 1
# baseline (speedup 1.0000x reference)
"""Trainium2 Bass kernel for nn_MoE_32332513804634.

MoE: 16 routed experts (top-6, softmax-then-bias routing) + dense shared
expert, T=4096 tokens, D=2048, H=1408, HS=2816, fp32.

Strategy (8 NeuronCores, SPMD):
  - Host computes the gate (cheap: 0.27 GFLOP) and per-expert token lists.
  - Expert parallelism with load-balanced segmentation: expert token lists
    are carved into pieces and packed into uniform per-core "segments"
    (1 big slot of cap 2048 + k small slots of cap 512 per core), so every
    core executes an identical instruction stream over 3584 token slots.
  - Each segment runs SwiGLU for its expert over its gathered tokens with
    bf16 matmuls accumulating in fp32 PSUM (halves HBM traffic vs f32r;
    the kernel is otherwise DMA-bound), the per-token combine weight
    applied as a per-partition DVE scale on the PSUM->SBUF copy.
  - Shared expert is tensor-parallel over its 2816 hidden dim (352 rows
    per core, padded to 384), same pipeline.
  - Host scatters segment outputs back to token rows, sums partials, and
    adds the second-layer biases (cw*b2 per expert, bs2 once) in fp32 --
    this removes ~240 augmented-row matmuls from the device.
"""

import sys
import numpy as np

sys.path.insert(0, "/opt/trn_rl_repo")

import concourse.bass as bass  # noqa: E402
import concourse.tile as tile  # noqa: E402
from concourse import bacc, mybir  # noqa: E402
from concourse.bass_utils import run_bass_kernel_spmd  # noqa: E402

T = 4096
D = 2048
H = 1408
E = 16
TOP_K = 6
HS = 2816
N_CORES = 8
HM = H // 128          # 11
KO = D // 128          # 16
HS_PAD = 384           # shared hidden shard (352) padded to 3*128
HMS = HS_PAD // 128    # 3
BIG_CAP = 2048
SMALL_CAP = 512
F32 = mybir.dt.float32
F32R = mybir.dt.float32r
BF16 = mybir.dt.bfloat16

# matmul operand dtype: bf16 halves HBM traffic (the kernel is DMA-bound in
# f32r) at ~2.5e-3 relative error; accumulation stays fp32 in PSUM.
MM_DT = BF16

_PROGRAM_CACHE: dict = {}


def _to_mm(a):
    if MM_DT == BF16:
        import ml_dtypes
        return np.ascontiguousarray(a).astype(ml_dtypes.bfloat16)
    return np.ascontiguousarray(a)


def _host_gate(xf, gate_w, gate_b):
    """Numpy replica of the reference gate. Returns cw [T, E] dense combine
    weights and per-expert token lists (ascending)."""
    scores = xf @ gate_w.T
    m = scores.max(axis=-1, keepdims=True)
    p = np.exp(scores - m, dtype=np.float32)
    probs = p / p.sum(axis=-1, keepdims=True)
    biased = probs + gate_b
    idx = np.argpartition(biased, E - TOP_K, axis=1)[:, E - TOP_K:]
    mask = np.zeros((xf.shape[0], E), dtype=bool)
    mask[np.arange(xf.shape[0])[:, None], idx] = True
    cw = np.where(mask, probs, 0.0).astype(np.float32)
    toks = [np.flatnonzero(mask[:, e]).astype(np.int64) for e in range(E)]
    return cw, toks


def _plan_segments(counts):
    """Carve expert token counts into pieces and pack into per-core slots.

    Returns (seg_caps, assignment) where seg_caps is the per-core slot
    capacity tuple and assignment[core][slot] = list of (expert, start, n)
    -- here each slot holds exactly one piece (expert, start offset into
    that expert's token list, piece length) or None for an empty slot.
    """
    order = np.argsort(counts)[::-1]
    bigs = []      # (expert, start, n) with n <= BIG_CAP
    smalls = []    # (expert, start, n) with n <= SMALL_CAP
    rema = []      # remainders to chop into smalls
    for i, e in enumerate(order):
        c = int(counts[e])
        if i < N_CORES:
            n = min(c, BIG_CAP)
            bigs.append((int(e), 0, n))
            if c > n:
                rema.append((int(e), n, c - n))
        else:
            rema.append((int(e), 0, c))
    for e, s0, rem in rema:
        o = 0
        while o < rem:
            n = min(SMALL_CAP, rem - o)
            smalls.append((e, s0 + o, n))
            o += n
    n_small_slots = -(-len(smalls) // N_CORES)  # ceil
    seg_caps = (BIG_CAP,) + (SMALL_CAP,) * n_small_slots
    assignment = []
    for c in range(N_CORES):
        slots = [bigs[c]]
        for s in range(n_small_slots):
            k = s * N_CORES + c
            slots.append(smalls[k] if k < len(smalls) else None)
        assignment.append(slots)
    return seg_caps, assignment


def _build_program(seg_caps):
    """Build the SPMD Bass program for the given per-core slot capacities."""
    nc = bacc.Bacc("TRN2", debug=False, num_devices=N_CORES)

    ins = {}
    outs = {}

    def din(name, shape, dt=MM_DT):
        ins[name] = nc.dram_tensor(name, list(shape), dt, kind="ExternalInput").ap()
        return ins[name]

    def dout(name, shape, dt=F32):
        outs[name] = nc.dram_tensor(name, list(shape), dt, kind="ExternalOutput").ap()
        return outs[name]

    for s, cap in enumerate(seg_caps):
        din(f"xg{s}", (D, cap))
        din(f"w1t{s}", (D, H))
        din(f"w3t{s}", (D, H))
        din(f"w2ta{s}", (H, D))
        din(f"b1_{s}", (128, HM), F32)
        din(f"b3_{s}", (128, HM), F32)
        din(f"scl{s}", (128, cap // 128), F32)
        dout(f"oe{s}", (cap, D))
    din("xt", (D, T))
    din("ws1s", (D, HS_PAD))
    din("ws3s", (D, HS_PAD))
    din("ws2sa", (HS_PAD, D))
    din("bs1", (128, HMS), F32)
    din("bs3", (128, HMS), F32)
    dout("zs", (T, D))

    with tile.TileContext(nc) as tc:
        with (
            tc.tile_pool(name="xpool", bufs=2) as xpool,
            tc.tile_pool(name="hpool", bufs=2) as hpool,
            tc.tile_pool(name="wcol", bufs=2) as wcol,
            tc.tile_pool(name="w2pool", bufs=2) as w2pool,
            tc.tile_pool(name="tmp", bufs=2) as tmp,
            tc.tile_pool(name="opool", bufs=3) as opool,
            tc.tile_pool(name="cpool", bufs=1) as cpool,
            tc.tile_pool(name="pp", bufs=2, space="PSUM") as pp,
        ):
            def mlp_segment(xg_ap, w1_ap, w3_ap, w2_ap, b1_ap, b3_ap,
                            scl_ap, out_ap, cap, n_hm, tag, scale_one):
                """One expert segment: out = scale * (swiglu(x) @ W2^T).
                Biases b2/bs2 are added on the host during the combine."""
                n_k2 = n_hm
                x3 = xg_ap.rearrange("(ko p) t -> p ko t", p=128)
                w1c3 = w1_ap.rearrange("(ko p) h -> p ko h", p=128)
                w3c3 = w3_ap.rearrange("(ko p) h -> p ko h", p=128)
                w23 = w2_ap.rearrange("(k p) d -> p k d", p=128)

                b1sb = cpool.tile([128, n_hm], F32, tag=f"b1{tag}")
                b3sb = cpool.tile([128, n_hm], F32, tag=f"b3{tag}")
                nc.sync.dma_start(b1sb[:], b1_ap)
                nc.sync.dma_start(b3sb[:], b3_ap)
                if not scale_one:
                    sclsb = cpool.tile([128, cap // 128], F32, tag=f"scl{tag}")
                    nc.sync.dma_start(sclsb[:], scl_ap)

                n_tc = cap // 512
                for t in range(n_tc):
                    xsb = xpool.tile([128, KO, 512], MM_DT, tag="xg")
                    nc.sync.dma_start(xsb[:], x3[:, :, t * 512:(t + 1) * 512])
                    hsb = hpool.tile([128, HM, 512], MM_DT, tag="h")
                    for hm in range(n_hm):
                        w1t_ = wcol.tile([128, KO, 128], MM_DT, tag="w1c")
                        nc.sync.dma_start(w1t_[:], w1c3[:, :, hm * 128:(hm + 1) * 128])
                        w3t_ = wcol.tile([128, KO, 128], MM_DT, tag="w3c")
                        nc.sync.dma_start(w3t_[:], w3c3[:, :, hm * 128:(hm + 1) * 128])
                        ps1 = pp.tile([128, 512], F32, tag="ph1")
                        for ko in range(KO):
                            nc.tensor.matmul(ps1[:], w1t_[:, ko, :], xsb[:, ko, :],
                                             start=(ko == 0), stop=(ko == KO - 1))
                        ps3 = pp.tile([128, 512], F32, tag="ph3")
                        for ko in range(KO):
                            nc.tensor.matmul(ps3[:], w3t_[:, ko, :], xsb[:, ko, :],
                                             start=(ko == 0), stop=(ko == KO - 1))
                        h1t = tmp.tile([128, 512], F32, tag="h1t")
                        nc.scalar.activation(h1t[:], ps1[:],
                                             mybir.ActivationFunctionType.Silu,
                                             bias=b1sb[:, hm:hm + 1])
                        h3t = tmp.tile([128, 512], F32, tag="h3t")
                        nc.scalar.activation(h3t[:], ps3[:],
                                             mybir.ActivationFunctionType.Identity,
                                             bias=b3sb[:, hm:hm + 1])
                        nc.vector.tensor_mul(hsb[:, hm, :], h1t[:], h3t[:])
                    # second matmul: out rows = tokens
                    for dm in range(4):
                        w2sb = w2pool.tile([128, n_k2, 512], MM_DT, tag="w2s")
                        nc.sync.dma_start(
                            w2sb[:], w23[:, :, dm * 512:(dm + 1) * 512])
                        for tch in range(4):
                            tok0 = t * 512 + tch * 128
                            ps2 = pp.tile([128, 512], F32, tag="po", bufs=4)
                            for k in range(n_k2):
                                lhsT = hsb[:, k, tch * 128:(tch + 1) * 128]
                                nc.tensor.matmul(ps2[:], lhsT,
                                                 w2sb[:, k, :],
                                                 start=(k == 0), stop=(k == n_k2 - 1))
                            osb = opool.tile([128, 512], F32, tag="osb")
                            if scale_one:
                                nc.vector.tensor_copy(osb[:], ps2[:])
                            else:
                                col = tok0 // 128
                                nc.vector.tensor_scalar_mul(
                                    osb[:], ps2[:], sclsb[:, col:col + 1])
                            nc.sync.dma_start(
                                out_ap[tok0:tok0 + 128, dm * 512:(dm + 1) * 512],
                                osb[:])

            for s, cap in enumerate(seg_caps):
                mlp_segment(ins[f"xg{s}"], ins[f"w1t{s}"], ins[f"w3t{s}"],
                            ins[f"w2ta{s}"], ins[f"b1_{s}"], ins[f"b3_{s}"],
                            ins[f"scl{s}"], outs[f"oe{s}"], cap, HM,
                            f"e{s}", False)
            # shared expert (hidden-sharded, all tokens, no combine scale)
            mlp_segment(ins["xt"], ins["ws1s"], ins["ws3s"], ins["ws2sa"],
                        ins["bs1"], ins["bs3"], None, outs["zs"], T, HMS,
                        "sh", True)

    nc.compile()
    return nc


def kernel(x, gate_w, gate_b, w1, b1, w2, b2, w3, b3,
           ws1, bs1, ws2, bs2, ws3, bs3):
    x = np.asarray(x, np.float32)
    xf = np.ascontiguousarray(x.reshape(-1, D))
    gate_w = np.asarray(gate_w, np.float32)
    gate_b = np.asarray(gate_b, np.float32)
    w1 = np.asarray(w1, np.float32)
    b1 = np.asarray(b1, np.float32)
    w2 = np.asarray(w2, np.float32)
    b2 = np.asarray(b2, np.float32)
    w3 = np.asarray(w3, np.float32)
    b3 = np.asarray(b3, np.float32)
    ws1 = np.asarray(ws1, np.float32)
    bs1 = np.asarray(bs1, np.float32)
    ws2 = np.asarray(ws2, np.float32)
    bs2 = np.asarray(bs2, np.float32)
    ws3 = np.asarray(ws3, np.float32)
    bs3 = np.asarray(bs3, np.float32)

    cw, toks = _host_gate(xf, gate_w, gate_b)
    counts = np.array([len(t) for t in toks])
    seg_caps, assignment = _plan_segments(counts)

    if seg_caps not in _PROGRAM_CACHE:
        _PROGRAM_CACHE[seg_caps] = _build_program(seg_caps)
    nc = _PROGRAM_CACHE[seg_caps]

    xT = np.ascontiguousarray(xf.T)  # [D, T]
    xT_mm = _to_mm(xT)

    # per-expert transposed weights (computed once, shared across pieces)
    w1t = {}
    w3t = {}
    w2ta = {}
    need = sorted({p[0] for slots in assignment for p in slots if p is not None})
    for e in need:
        w1t[e] = _to_mm(w1[e].T)
        w3t[e] = _to_mm(w3[e].T)
        w2ta[e] = _to_mm(w2[e].T)

    # shared expert shards
    hs_per = HS // N_CORES  # 352

    in_maps = []
    for c in range(N_CORES):
        m = {}
        for s, cap in enumerate(seg_caps):
            piece = assignment[c][s]
            xg = np.zeros((D, cap), np.float32)
            scl = np.zeros(cap, np.float32)
            if piece is None:
                e = need[0]
                m[f"w1t{s}"] = w1t[e]
                m[f"w3t{s}"] = w3t[e]
                m[f"w2ta{s}"] = w2ta[e]
                m[f"b1_{s}"] = np.zeros((128, HM), np.float32)
                m[f"b3_{s}"] = np.zeros((128, HM), np.float32)
            else:
                e, s0, n = piece
                tk = toks[e][s0:s0 + n]
                xg[:, :n] = xT[:, tk]
                scl[:n] = cw[tk, e]
                m[f"w1t{s}"] = w1t[e]
                m[f"w3t{s}"] = w3t[e]
                m[f"w2ta{s}"] = w2ta[e]
                m[f"b1_{s}"] = np.ascontiguousarray(
                    b1[e].reshape(HM, 128).T)
                m[f"b3_{s}"] = np.ascontiguousarray(
                    b3[e].reshape(HM, 128).T)
            m[f"xg{s}"] = _to_mm(xg)
            m[f"scl{s}"] = np.ascontiguousarray(
                scl.reshape(cap // 128, 128).T)
        # shared shard
        r0 = c * hs_per
        ws1p = np.zeros((D, HS_PAD), np.float32)
        ws1p[:, :hs_per] = ws1[r0:r0 + hs_per].T
        ws3p = np.zeros((D, HS_PAD), np.float32)
        ws3p[:, :hs_per] = ws3[r0:r0 + hs_per].T
        ws2a = np.zeros((HS_PAD, D), np.float32)
        ws2a[:hs_per] = ws2[:, r0:r0 + hs_per].T
        bs1p = np.zeros(HS_PAD, np.float32)
        bs1p[:hs_per] = bs1[r0:r0 + hs_per]
        bs3p = np.zeros(HS_PAD, np.float32)
        bs3p[:hs_per] = bs3[r0:r0 + hs_per]
        m["xt"] = xT_mm
        m["ws1s"] = _to_mm(ws1p)
        m["ws3s"] = _to_mm(ws3p)
        m["ws2sa"] = _to_mm(ws2a)
        m["bs1"] = np.ascontiguousarray(bs1p.reshape(HMS, 128).T)
        m["bs3"] = np.ascontiguousarray(bs3p.reshape(HMS, 128).T)
        in_maps.append(m)

    res = run_bass_kernel_spmd(nc, in_maps, list(range(N_CORES)))

    # host combine: scatter segment outputs + sum shared partials
    y = np.zeros((T, D), np.float32)
    for c in range(N_CORES):
        for s, cap in enumerate(seg_caps):
            piece = assignment[c][s]
            if piece is None:
                continue
            e, s0, n = piece
            tk = toks[e][s0:s0 + n]
            y[tk] += res.results[c][f"oe{s}"][:n]
            y[tk] += cw[tk, e][:, None] * b2[e][None, :]
        y += res.results[c]["zs"]
    y += bs2[None, :]
    return y.reshape(x.shape).astype(np.float32)



# revision 2
# speedup vs baseline: 1.2836x; 1.2836x over previous
"""Trainium2 Bass kernel for nn_MoE_32332513804634.

MoE: 16 routed experts (top-6, softmax-then-bias routing) + dense shared
expert, T=4096 tokens, D=2048, H=1408, HS=2816, fp32.

Strategy (8 NeuronCores, SPMD):
  - Host computes the gate (cheap) and per-expert token lists.
  - Expert parallelism as a flat list of variable-width "jobs" (128-token
    granularity, up to 1024 tokens each). All cores run the identical
    job-width schedule; the host binds each (core, job) to an arbitrary
    expert piece, so load balance is a pure host-side packing problem.
    A small schedule search minimizes padded capacity (~3200 tokens/core
    vs the 3072 ideal).
  - Weights are streamed per job in DMA-friendly pre-tiled DRAM layouts
    (per-partition contiguous runs of 8-22KB), loaded once per job and
    reused across the job's 512-token chunks.
  - Shared expert is token-parallel: each core runs its 512 tokens
    through the full HS=2816 hidden dim (22 even 128-subtiles, no pad).
  - bf16 matmuls accumulate in fp32 PSUM; per-token combine weights are
    applied as a per-partition DVE scale on the PSUM->SBUF copy; the
    second-layer biases (cw*b2, bs2) are added on the host.
"""

import sys
import numpy as np

sys.path.insert(0, "/opt/trn_rl_repo")

import concourse.bass as bass  # noqa: E402
import concourse.tile as tile  # noqa: E402
from concourse import bacc, mybir  # noqa: E402
from concourse.bass_utils import run_bass_kernel_spmd  # noqa: E402

T = 4096
D = 2048
H = 1408
E = 16
TOP_K = 6
HS = 2816
N_CORES = 8
HM = H // 128           # 11
KO = D // 128           # 16
HMS = HS // 128         # 22
TS = T // N_CORES       # 512 shared-expert tokens per core
F32 = mybir.dt.float32
BF16 = mybir.dt.bfloat16
MM_DT = BF16

_PROGRAM_CACHE: dict = {}


def _to_mm(a):
    import ml_dtypes
    return np.ascontiguousarray(a).astype(ml_dtypes.bfloat16)


def _host_gate(xf, gate_w, gate_b):
    """Numpy replica of the reference gate. Returns cw [T, E] dense combine
    weights and per-expert token lists (ascending)."""
    scores = xf @ gate_w.T
    m = scores.max(axis=-1, keepdims=True)
    p = np.exp(scores - m, dtype=np.float32)
    probs = p / p.sum(axis=-1, keepdims=True)
    biased = probs + gate_b
    idx = np.argpartition(biased, E - TOP_K, axis=1)[:, E - TOP_K:]
    mask = np.zeros((xf.shape[0], E), dtype=bool)
    mask[np.arange(xf.shape[0])[:, None], idx] = True
    cw = np.where(mask, probs, 0.0).astype(np.float32)
    toks = [np.flatnonzero(mask[:, e]).astype(np.int64) for e in range(E)]
    return cw, toks


def _pack(counts, widths, prefer_full):
    """Assign expert pieces to an 8 x len(widths) grid of cells.

    Cells are processed in descending width. Each cell takes one
    contiguous piece of one expert's token list. Returns
    assignment[core][j] = (expert, start, n) | None, or None if some
    tokens could not be placed.
    """
    cells = []
    for j, w in enumerate(widths):
        for c in range(N_CORES):
            cells.append((w, c, j))
    cells.sort(key=lambda x: -x[0])
    rem = np.array(counts, dtype=np.int64).copy()
    starts = np.zeros(E, dtype=np.int64)
    asg = [[None] * len(widths) for _ in range(N_CORES)]
    for w, c, j in cells:
        if prefer_full:
            big = np.flatnonzero(rem >= w)
            e = big[np.argmax(rem[big])] if len(big) else int(np.argmax(rem))
        else:
            e = int(np.argmax(rem))
        if rem[e] <= 0:
            continue
        n = int(min(rem[e], w))
        asg[c][j] = (int(e), int(starts[e]), n)
        starts[e] += n
        rem[e] -= n
    if rem.sum() > 0:
        return None
    return asg


def _plan_jobs(counts):
    """Pick the per-core job-width schedule (identical across cores) and
    the piece assignment. Minimizes padded token capacity, then job count."""
    total = int(np.sum(counts))
    percore_lo = -(-total // N_CORES)
    cands = []
    for n1024 in range(0, 4):
        for n512 in range(0, 8):
            for n384 in range(0, 4):
                for n256 in range(0, 4):
                    for n128 in range(0, 3):
                        cap = (1024 * n1024 + 512 * n512 + 384 * n384
                               + 256 * n256 + 128 * n128)
                        if not (percore_lo <= cap <= percore_lo + 640):
                            continue
                        w = ([1024] * n1024 + [512] * n512 + [384] * n384
                             + [256] * n256 + [128] * n128)
                        njobs = len(w)
                        cands.append((cap, njobs, -min(w), tuple(w)))
    cands.sort()
    for cap, njobs, _, widths in cands:
        for prefer_full in (True, False):
            asg = _pack(counts, widths, prefer_full)
            if asg is not None:
                # ascending widths: small jobs first for a short lead-in
                order = np.argsort([w for w in widths], kind="stable")
                widths_o = tuple(int(widths[i]) for i in order)
                asg_o = [[asg[c][int(i)] for i in order] for c in range(N_CORES)]
                return widths_o, asg_o
    raise RuntimeError("no feasible job schedule found")


def _build_program(widths):
    """Build the SPMD Bass program for the given per-core job widths."""
    nc = bacc.Bacc("TRN2", debug=False, num_devices=N_CORES)

    ins = {}
    outs = {}

    def din(name, shape, dt=MM_DT):
        ins[name] = nc.dram_tensor(name, list(shape), dt, kind="ExternalInput").ap()
        return ins[name]

    def dout(name, shape, dt=F32):
        outs[name] = nc.dram_tensor(name, list(shape), dt, kind="ExternalOutput").ap()
        return outs[name]

    for j, w in enumerate(widths):
        din(f"xg{j}", (128, KO, w))
        din(f"w13_{j}", (HM * 128, KO, 256))
        din(f"w2_{j}", (4 * 128, HM, 512))
        din(f"b13_{j}", (128, 2 * HM), F32)
        din(f"scl{j}", (128, w // 128), F32)
        dout(f"oe{j}", (w, D))
    din("xs", (128, KO, TS))
    din("ws13", (HMS * 128, KO, 256))
    din("ws2", (4 * 128, HMS, 512))
    din("bs13", (128, 2 * HMS), F32)
    dout("zs", (TS, D))

    with tile.TileContext(nc) as tc:
        with (
            tc.tile_pool(name="xpool", bufs=2) as xpool,
            tc.tile_pool(name="hpool", bufs=2) as hpool,
            tc.tile_pool(name="wcol", bufs=2) as wcol,
            tc.tile_pool(name="w2pool", bufs=2) as w2pool,
            tc.tile_pool(name="tmp", bufs=2) as tmp,
            tc.tile_pool(name="opool", bufs=3) as opool,
            tc.tile_pool(name="cpool", bufs=1) as cpool,
            tc.tile_pool(name="pp", bufs=2, space="PSUM") as pp,
        ):
            def mlp_job(xg_ap, w13_ap, w2_ap, b13_ap, scl_ap, out_ap,
                        w, n_hm, tag, scale_one):
                """One job: out = scale * (swiglu(x) @ W2^T), W2 bias on host."""
                b13sb = cpool.tile([128, 2 * n_hm], F32, tag=f"b{tag}")
                nc.sync.dma_start(b13sb[:], b13_ap)
                if not scale_one:
                    sclsb = cpool.tile([128, w // 128], F32, tag=f"s{tag}")
                    nc.sync.dma_start(sclsb[:], scl_ap)
                chunks = []
                o = 0
                while o < w:
                    cw_ = min(512, w - o)
                    chunks.append((o, cw_))
                    o += cw_

                xsb = xpool.tile([128, KO, w], MM_DT, tag="xg")
                nc.sync.dma_start(xsb[:], xg_ap)
                hsb = hpool.tile([128, n_hm, w], MM_DT, tag="h")
                for hm in range(n_hm):
                    wsb = wcol.tile([128, KO, 256], MM_DT, tag="w13")
                    nc.sync.dma_start(wsb[:], w13_ap[hm * 128:(hm + 1) * 128])
                    for (c0, cw_) in chunks:
                        ps1 = pp.tile([128, cw_], F32, tag="ph1")
                        for ko in range(KO):
                            nc.tensor.matmul(ps1[:], wsb[:, ko, 0:128],
                                             xsb[:, ko, c0:c0 + cw_],
                                             start=(ko == 0), stop=(ko == KO - 1))
                        ps3 = pp.tile([128, cw_], F32, tag="ph3")
                        for ko in range(KO):
                            nc.tensor.matmul(ps3[:], wsb[:, ko, 128:256],
                                             xsb[:, ko, c0:c0 + cw_],
                                             start=(ko == 0), stop=(ko == KO - 1))
                        h1t = tmp.tile([128, cw_], F32, tag="h1t")
                        nc.scalar.activation(h1t[:], ps1[:],
                                             mybir.ActivationFunctionType.Silu,
                                             bias=b13sb[:, hm:hm + 1])
                        h3t = tmp.tile([128, cw_], F32, tag="h3t")
                        nc.scalar.activation(h3t[:], ps3[:],
                                             mybir.ActivationFunctionType.Identity,
                                             bias=b13sb[:, n_hm + hm:n_hm + hm + 1])
                        nc.vector.tensor_mul(hsb[:, hm, c0:c0 + cw_],
                                             h1t[:], h3t[:])
                for dm in range(4):
                    w2sb = w2pool.tile([128, n_hm, 512], MM_DT, tag="w2s")
                    nc.sync.dma_start(w2sb[:], w2_ap[dm * 128:(dm + 1) * 128])
                    for tch in range(w // 128):
                        ps2 = pp.tile([128, 512], F32, tag="po", bufs=4)
                        for k in range(n_hm):
                            nc.tensor.matmul(ps2[:],
                                             hsb[:, k, tch * 128:(tch + 1) * 128],
                                             w2sb[:, k, :],
                                             start=(k == 0), stop=(k == n_hm - 1))
                        osb = opool.tile([128, 512], F32, tag="osb")
                        if scale_one:
                            nc.vector.tensor_copy(osb[:], ps2[:])
                        else:
                            nc.vector.tensor_scalar_mul(
                                osb[:], ps2[:], sclsb[:, tch:tch + 1])
                        nc.sync.dma_start(
                            out_ap[tch * 128:(tch + 1) * 128,
                                   dm * 512:(dm + 1) * 512],
                            osb[:])

            for j, w in enumerate(widths):
                mlp_job(ins[f"xg{j}"], ins[f"w13_{j}"], ins[f"w2_{j}"],
                        ins[f"b13_{j}"], ins[f"scl{j}"], outs[f"oe{j}"],
                        w, HM, f"e{j}", False)
            # shared expert: this core's 512 tokens through full HS
            mlp_job(ins["xs"], ins["ws13"], ins["ws2"], ins["bs13"], None,
                    outs["zs"], TS, HMS, "sh", True)

    nc.compile()
    return nc


def _tile_w13(w1e, w3e, n_hm):
    a = w1e.reshape(n_hm, 128, KO, 128).transpose(0, 3, 2, 1)
    b = w3e.reshape(n_hm, 128, KO, 128).transpose(0, 3, 2, 1)
    cat = np.concatenate([a, b], axis=3)           # [n_hm, 128, KO, 256]
    return _to_mm(cat.reshape(n_hm * 128, KO, 256))


def _tile_w2(w2e, n_hm):
    a = w2e.reshape(4, 512, n_hm, 128).transpose(0, 3, 2, 1)
    return _to_mm(a.reshape(4 * 128, n_hm, 512))   # [4*128, n_hm, 512]


def _tile_b13(b1e, b3e, n_hm):
    return np.ascontiguousarray(np.concatenate(
        [b1e.reshape(n_hm, 128).T, b3e.reshape(n_hm, 128).T],
        axis=1).astype(np.float32))                # [128, 2*n_hm]


def kernel(x, gate_w, gate_b, w1, b1, w2, b2, w3, b3,
           ws1, bs1, ws2, bs2, ws3, bs3):
    x = np.asarray(x, np.float32)
    xf = np.ascontiguousarray(x.reshape(-1, D))
    gate_w = np.asarray(gate_w, np.float32)
    gate_b = np.asarray(gate_b, np.float32)
    w1 = np.asarray(w1, np.float32)
    b1 = np.asarray(b1, np.float32)
    w2 = np.asarray(w2, np.float32)
    b2 = np.asarray(b2, np.float32)
    w3 = np.asarray(w3, np.float32)
    b3 = np.asarray(b3, np.float32)
    ws1 = np.asarray(ws1, np.float32)
    bs1 = np.asarray(bs1, np.float32)
    ws2 = np.asarray(ws2, np.float32)
    bs2 = np.asarray(bs2, np.float32)
    ws3 = np.asarray(ws3, np.float32)
    bs3 = np.asarray(bs3, np.float32)

    cw, toks = _host_gate(xf, gate_w, gate_b)
    counts = np.array([len(t) for t in toks])
    widths, asg = _plan_jobs(counts)

    if widths not in _PROGRAM_CACHE:
        _PROGRAM_CACHE[widths] = _build_program(widths)
    nc = _PROGRAM_CACHE[widths]

    xT3 = np.ascontiguousarray(xf.T.reshape(KO, 128, T))   # [KO, 128, T]

    need = sorted({p[0] for slots in asg for p in slots if p is not None})
    w13t = {e: _tile_w13(w1[e], w3[e], HM) for e in need}
    w2t = {e: _tile_w2(w2[e], HM) for e in need}
    b13t = {e: _tile_b13(b1[e], b3[e], HM) for e in need}
    ws13t = _tile_w13(ws1, ws3, HMS)
    ws2t = _tile_w2(ws2, HMS)
    bs13t = _tile_b13(bs1, bs3, HMS)
    zero_b13 = np.zeros((128, 2 * HM), np.float32)

    in_maps = []
    for c in range(N_CORES):
        m = {}
        for j, w in enumerate(widths):
            piece = asg[c][j]
            xg = np.zeros((128, KO, w), np.float32)
            scl = np.zeros(w, np.float32)
            if piece is None:
                e0 = need[0]
                m[f"w13_{j}"] = w13t[e0]
                m[f"w2_{j}"] = w2t[e0]
                m[f"b13_{j}"] = zero_b13
            else:
                e, s0, n = piece
                tk = toks[e][s0:s0 + n]
                xg[:, :, :n] = xT3[:, :, tk].transpose(1, 0, 2)
                scl[:n] = cw[tk, e]
                m[f"w13_{j}"] = w13t[e]
                m[f"w2_{j}"] = w2t[e]
                m[f"b13_{j}"] = b13t[e]
            m[f"xg{j}"] = _to_mm(xg)
            m[f"scl{j}"] = np.ascontiguousarray(scl.reshape(w // 128, 128).T)
        m["xs"] = _to_mm(xT3[:, :, c * TS:(c + 1) * TS].transpose(1, 0, 2))
        m["ws13"] = ws13t
        m["ws2"] = ws2t
        m["bs13"] = bs13t
        in_maps.append(m)

    res = run_bass_kernel_spmd(nc, in_maps, list(range(N_CORES)))

    # host combine: scatter job outputs + concat shared partials
    y = np.zeros((T, D), np.float32)
    for c in range(N_CORES):
        for j, w in enumerate(widths):
            piece = asg[c][j]
            if piece is None:
                continue
            e, s0, n = piece
            tk = toks[e][s0:s0 + n]
            y[tk] += res.results[c][f"oe{j}"][:n]
            y[tk] += cw[tk, e][:, None] * b2[e][None, :]
        y[c * TS:(c + 1) * TS] += res.results[c]["zs"]
    y += bs2[None, :]
    return y.reshape(x.shape).astype(np.float32)


# revision 8
# speedup vs baseline: 1.3737x; 1.0702x over previous
"""Trainium2 Bass kernel for nn_MoE_32332513804634.

MoE: 16 routed experts (top-6, softmax-then-bias routing) + dense shared
expert, T=4096 tokens, D=2048, H=1408, HS=2816, fp32.

Strategy (8 NeuronCores, SPMD):
  - Host computes the gate (cheap) and per-expert token lists.
  - Expert parallelism as a flat list of variable-width "jobs" (128-token
    granularity, up to 1024 tokens each). All cores run the identical
    job-width schedule; the host binds each (core, job) to an arbitrary
    expert piece, so load balance is a pure host-side packing problem.
    A small schedule search minimizes padded capacity (~3200 tokens/core
    vs the 3072 ideal).
  - Weights are streamed per job in DMA-friendly pre-tiled DRAM layouts
    (per-partition contiguous runs of 8-22KB), loaded once per job and
    reused across the job's 512-token chunks.
  - Shared expert is token-parallel: each core runs its 512 tokens
    through the full HS=2816 hidden dim (22 even 128-subtiles, no pad).
  - bf16 matmuls accumulate in fp32 PSUM; per-token combine weights are
    applied as a per-partition DVE scale on the PSUM->SBUF copy; the
    second-layer biases (cw*b2, bs2) are added on the host.
"""

import sys
import numpy as np

sys.path.insert(0, "/opt/trn_rl_repo")

import concourse.bass as bass  # noqa: E402
import concourse.tile as tile  # noqa: E402
from concourse import bacc, mybir  # noqa: E402
from concourse.bass_utils import run_bass_kernel_spmd  # noqa: E402

T = 4096
D = 2048
H = 1408
E = 16
TOP_K = 6
HS = 2816
N_CORES = 8
HM = H // 128           # 11
KO = D // 128           # 16
HMS = HS // 128         # 22
TS = T // N_CORES       # 512 shared-expert tokens per core
F32 = mybir.dt.float32
BF16 = mybir.dt.bfloat16
MM_DT = BF16

_PROGRAM_CACHE: dict = {}


def _to_mm(a):
    import ml_dtypes
    return np.ascontiguousarray(a).astype(ml_dtypes.bfloat16)


def _host_gate(xf, gate_w, gate_b):
    """Numpy replica of the reference gate. Returns cw [T, E] dense combine
    weights and per-expert token lists (ascending)."""
    scores = xf @ gate_w.T
    m = scores.max(axis=-1, keepdims=True)
    p = np.exp(scores - m, dtype=np.float32)
    probs = p / p.sum(axis=-1, keepdims=True)
    biased = probs + gate_b
    idx = np.argpartition(biased, E - TOP_K, axis=1)[:, E - TOP_K:]
    mask = np.zeros((xf.shape[0], E), dtype=bool)
    mask[np.arange(xf.shape[0])[:, None], idx] = True
    cw = np.where(mask, probs, 0.0).astype(np.float32)
    toks = [np.flatnonzero(mask[:, e]).astype(np.int64) for e in range(E)]
    return cw, toks


def _group_partition(counts, widths):
    """Partition the 8x copies of `widths` cells into one cell-group per
    expert with group capacity >= count (global slack bounded by the
    schedule's total padding). Returns groups[e] = list of cell widths,
    or None. Backtracking over experts in descending count order."""
    distinct = sorted(set(widths), reverse=True)
    avail = {w: 8 * widths.count(w) for w in distinct}
    total_cap = sum(w * n for w, n in avail.items())
    order = np.argsort(-np.asarray(counts))
    slack = total_cap - int(np.sum(counts))
    if slack < 0:
        return None

    def combos(target, limit):
        """Cell-width multisets (from current avail) with sum in
        [target, target+limit], cheapest (least waste, fewest) first."""
        out = []

        def rec(i, acc, cap):
            if target <= cap <= target + limit:
                out.append(list(acc))
                return  # adding more cells only wastes
            if i >= len(distinct) or len(out) >= 400:
                return
            w = distinct[i]
            max_n = min(avail[w], -(-(target - cap) // w))
            for n in range(max_n, -1, -1):
                if cap + n * w > target + limit:
                    continue
                acc.extend([w] * n)
                rec(i + 1, acc, cap + n * w)
                if n:
                    del acc[-n:]

        rec(0, [], 0)
        out.sort(key=lambda g: (sum(g), len(g)))
        return out[:80]

    groups = [None] * E
    budget = [20000]

    def solve(k, slack_left):
        if k == len(order):
            return True
        if budget[0] <= 0:
            return False
        budget[0] -= 1
        e = int(order[k])
        target = int(counts[e])
        for g in combos(target, slack_left):
            for w in g:
                avail[w] -= 1
            groups[e] = g
            if solve(k + 1, slack_left - (sum(g) - target)):
                return True
            for w in g:
                avail[w] += 1
            groups[e] = None
        return False

    if not solve(0, slack):
        return None
    return groups


def _pack(counts, widths):
    """Assign expert pieces to an 8 x len(widths) grid of cells via the
    group-partition solver. Returns assignment[core][j] = (expert, start,
    n) | None, or None if infeasible."""
    groups = _group_partition(counts, widths)
    if groups is None:
        return None
    # free cells per width: list of (core, j)
    free = {}
    for j, w in enumerate(widths):
        for c in range(N_CORES):
            free.setdefault(w, []).append((c, j))
    asg = [[None] * len(widths) for _ in range(N_CORES)]
    for e in range(E):
        g = sorted(groups[e], reverse=True)
        rem = int(counts[e])
        start = 0
        for w in g:
            c, j = free[w].pop()
            n = min(rem, w)
            if n > 0:
                asg[c][j] = (int(e), start, int(n))
            start += n
            rem -= n
        assert rem == 0
    return asg


def _plan_jobs(counts):
    """Pick the per-core job-width schedule (identical across cores) and
    the piece assignment. Minimizes padded token capacity, then job
    count, preferring wider minimum job width (DMA-friendlier)."""
    total = int(np.sum(counts))
    percore_lo = -(-total // N_CORES)
    wvocab = [1024, 896, 768, 640, 512, 384, 256, 128]
    cands = []

    def gen(i, acc, cap, ndist):
        if percore_lo <= cap <= percore_lo + 640 and acc:
            cands.append((cap, len(acc), -min(acc), tuple(acc)))
        if i >= len(wvocab) or cap > percore_lo + 640 or len(acc) >= 9:
            return
        w = wvocab[i]
        for n in range(0, min(8, (percore_lo + 640 - cap) // w) + 1):
            nd = ndist + (1 if n else 0)
            if nd > 3:
                break
            gen(i + 1, acc + [w] * n, cap + n * w, nd)

    gen(0, [], 0, 0)
    cands = sorted(set(cands))
    for cap, njobs, _, widths in cands:
        asg = _pack(counts, widths)
        if asg is not None:
            order = np.argsort([w for w in widths], kind="stable")[::-1]
            widths_o = tuple(int(widths[i]) for i in order)
            asg_o = [[asg[c][int(i)] for i in order] for c in range(N_CORES)]
            return widths_o, asg_o
    raise RuntimeError("no feasible job schedule found")


def _build_program(widths):
    """Build the SPMD Bass program for the given per-core job widths."""
    nc = bacc.Bacc("TRN2", debug=False, num_devices=N_CORES)

    ins = {}
    outs = {}

    def din(name, shape, dt=MM_DT):
        ins[name] = nc.dram_tensor(name, list(shape), dt, kind="ExternalInput").ap()
        return ins[name]

    def dout(name, shape, dt=F32):
        outs[name] = nc.dram_tensor(name, list(shape), dt, kind="ExternalOutput").ap()
        return outs[name]

    for j, w in enumerate(widths):
        din(f"xg{j}", (128, KO, w))
        din(f"w13_{j}", (HM * 128, KO, 256))
        din(f"w2_{j}", (4 * 128, HM, 512))
        din(f"b13_{j}", (128, 2 * HM), F32)
        din(f"scl{j}", (128, w // 128), F32)
        dout(f"oe{j}", (w, D))
    din("xs", (128, KO, TS))
    din("ws13", (HMS * 128, KO, 256))
    din("ws2", (4 * 128, HMS, 512))
    din("bs13", (128, 2 * HMS), F32)
    dout("zs", (TS, D))

    with tile.TileContext(nc) as tc:
        with (
            tc.tile_pool(name="xpool", bufs=2) as xpool,
            tc.tile_pool(name="hpool", bufs=2) as hpool,
            tc.tile_pool(name="wcol", bufs=2) as wcol,
            tc.tile_pool(name="w2pool", bufs=2) as w2pool,
            tc.tile_pool(name="tmp", bufs=2) as tmp,
            tc.tile_pool(name="opool", bufs=3) as opool,
            tc.tile_pool(name="cpool", bufs=1) as cpool,
            tc.tile_pool(name="pp", bufs=2, space="PSUM") as pp,
        ):
            def mlp_job(xg_ap, w13_ap, w2_ap, b13_ap, scl_ap, out_ap,
                        w, n_hm, tag, scale_one):
                """One job: out = scale * (swiglu(x) @ W2^T), W2 bias on host."""
                b13sb = cpool.tile([128, 2 * n_hm], F32, tag=f"b{tag}")
                nc.sync.dma_start(b13sb[:], b13_ap)
                if not scale_one:
                    sclsb = cpool.tile([128, w // 128], F32, tag=f"s{tag}")
                    nc.sync.dma_start(sclsb[:], scl_ap)
                chunks = []
                o = 0
                while o < w:
                    cw_ = min(512, w - o)
                    chunks.append((o, cw_))
                    o += cw_

                xsb = xpool.tile([128, KO, w], MM_DT, tag="xg")
                nc.sync.dma_start(xsb[:], xg_ap)
                hsb = hpool.tile([128, n_hm, w], MM_DT, tag="h")
                for hm in range(n_hm):
                    wsb = wcol.tile([128, KO, 256], MM_DT, tag="w13")
                    nc.sync.dma_start(wsb[:], w13_ap[hm * 128:(hm + 1) * 128])
                    for (c0, cw_) in chunks:
                        ps1 = pp.tile([128, cw_], F32, tag="ph1")
                        for ko in range(KO):
                            nc.tensor.matmul(ps1[:], wsb[:, ko, 0:128],
                                             xsb[:, ko, c0:c0 + cw_],
                                             start=(ko == 0), stop=(ko == KO - 1))
                        ps3 = pp.tile([128, cw_], F32, tag="ph3")
                        for ko in range(KO):
                            nc.tensor.matmul(ps3[:], wsb[:, ko, 128:256],
                                             xsb[:, ko, c0:c0 + cw_],
                                             start=(ko == 0), stop=(ko == KO - 1))
                        h1t = tmp.tile([128, cw_], F32, tag="h1t")
                        nc.scalar.activation(h1t[:], ps1[:],
                                             mybir.ActivationFunctionType.Silu,
                                             bias=b13sb[:, hm:hm + 1])
                        h3t = tmp.tile([128, cw_], F32, tag="h3t")
                        nc.scalar.activation(h3t[:], ps3[:],
                                             mybir.ActivationFunctionType.Identity,
                                             bias=b13sb[:, n_hm + hm:n_hm + hm + 1])
                        nc.vector.tensor_mul(hsb[:, hm, c0:c0 + cw_],
                                             h1t[:], h3t[:])
                for dm in range(4):
                    w2sb = w2pool.tile([128, n_hm, 512], MM_DT, tag="w2s")
                    nc.sync.dma_start(w2sb[:], w2_ap[dm * 128:(dm + 1) * 128])
                    for tch in range(w // 128):
                        ps2 = pp.tile([128, 512], F32, tag="po", bufs=4)
                        for k in range(n_hm):
                            nc.tensor.matmul(ps2[:],
                                             hsb[:, k, tch * 128:(tch + 1) * 128],
                                             w2sb[:, k, :],
                                             start=(k == 0), stop=(k == n_hm - 1))
                        osb = opool.tile([128, 512], F32, tag="osb")
                        if scale_one:
                            nc.vector.tensor_copy(osb[:], ps2[:])
                        else:
                            nc.vector.tensor_scalar_mul(
                                osb[:], ps2[:], sclsb[:, tch:tch + 1])
                        nc.sync.dma_start(
                            out_ap[tch * 128:(tch + 1) * 128,
                                   dm * 512:(dm + 1) * 512],
                            osb[:])

            # shared expert first: heaviest weight stream gets the
            # DMA-idle window at kernel start
            mlp_job(ins["xs"], ins["ws13"], ins["ws2"], ins["bs13"], None,
                    outs["zs"], TS, HMS, "sh", True)
            for j, w in enumerate(widths):
                mlp_job(ins[f"xg{j}"], ins[f"w13_{j}"], ins[f"w2_{j}"],
                        ins[f"b13_{j}"], ins[f"scl{j}"], outs[f"oe{j}"],
                        w, HM, f"e{j}", False)

    nc.compile()
    return nc


def _tile_w13(w1e, w3e, n_hm):
    a = w1e.reshape(n_hm, 128, KO, 128).transpose(0, 3, 2, 1)
    b = w3e.reshape(n_hm, 128, KO, 128).transpose(0, 3, 2, 1)
    cat = np.concatenate([a, b], axis=3)           # [n_hm, 128, KO, 256]
    return _to_mm(cat.reshape(n_hm * 128, KO, 256))


def _tile_w2(w2e, n_hm):
    a = w2e.reshape(4, 512, n_hm, 128).transpose(0, 3, 2, 1)
    return _to_mm(a.reshape(4 * 128, n_hm, 512))   # [4*128, n_hm, 512]


def _tile_b13(b1e, b3e, n_hm):
    return np.ascontiguousarray(np.concatenate(
        [b1e.reshape(n_hm, 128).T, b3e.reshape(n_hm, 128).T],
        axis=1).astype(np.float32))                # [128, 2*n_hm]


def kernel(x, gate_w, gate_b, w1, b1, w2, b2, w3, b3,
           ws1, bs1, ws2, bs2, ws3, bs3):
    x = np.asarray(x, np.float32)
    xf = np.ascontiguousarray(x.reshape(-1, D))
    gate_w = np.asarray(gate_w, np.float32)
    gate_b = np.asarray(gate_b, np.float32)
    w1 = np.asarray(w1, np.float32)
    b1 = np.asarray(b1, np.float32)
    w2 = np.asarray(w2, np.float32)
    b2 = np.asarray(b2, np.float32)
    w3 = np.asarray(w3, np.float32)
    b3 = np.asarray(b3, np.float32)
    ws1 = np.asarray(ws1, np.float32)
    bs1 = np.asarray(bs1, np.float32)
    ws2 = np.asarray(ws2, np.float32)
    bs2 = np.asarray(bs2, np.float32)
    ws3 = np.asarray(ws3, np.float32)
    bs3 = np.asarray(bs3, np.float32)

    cw, toks = _host_gate(xf, gate_w, gate_b)
    counts = np.array([len(t) for t in toks])
    widths, asg = _plan_jobs(counts)

    if widths not in _PROGRAM_CACHE:
        _PROGRAM_CACHE[widths] = _build_program(widths)
    nc = _PROGRAM_CACHE[widths]

    xT3 = np.ascontiguousarray(xf.T.reshape(KO, 128, T))   # [KO, 128, T]

    need = sorted({p[0] for slots in asg for p in slots if p is not None})
    w13t = {e: _tile_w13(w1[e], w3[e], HM) for e in need}
    w2t = {e: _tile_w2(w2[e], HM) for e in need}
    b13t = {e: _tile_b13(b1[e], b3[e], HM) for e in need}
    ws13t = _tile_w13(ws1, ws3, HMS)
    ws2t = _tile_w2(ws2, HMS)
    bs13t = _tile_b13(bs1, bs3, HMS)
    zero_b13 = np.zeros((128, 2 * HM), np.float32)

    in_maps = []
    for c in range(N_CORES):
        m = {}
        for j, w in enumerate(widths):
            piece = asg[c][j]
            xg = np.zeros((128, KO, w), np.float32)
            scl = np.zeros(w, np.float32)
            if piece is None:
                e0 = need[0]
                m[f"w13_{j}"] = w13t[e0]
                m[f"w2_{j}"] = w2t[e0]
                m[f"b13_{j}"] = zero_b13
            else:
                e, s0, n = piece
                tk = toks[e][s0:s0 + n]
                xg[:, :, :n] = xT3[:, :, tk].transpose(1, 0, 2)
                scl[:n] = cw[tk, e]
                m[f"w13_{j}"] = w13t[e]
                m[f"w2_{j}"] = w2t[e]
                m[f"b13_{j}"] = b13t[e]
            m[f"xg{j}"] = _to_mm(xg)
            m[f"scl{j}"] = np.ascontiguousarray(scl.reshape(w // 128, 128).T)
        m["xs"] = _to_mm(xT3[:, :, c * TS:(c + 1) * TS].transpose(1, 0, 2))
        m["ws13"] = ws13t
        m["ws2"] = ws2t
        m["bs13"] = bs13t
        in_maps.append(m)

    res = run_bass_kernel_spmd(nc, in_maps, list(range(N_CORES)))

    # host combine: scatter job outputs + concat shared partials
    y = np.zeros((T, D), np.float32)
    for c in range(N_CORES):
        for j, w in enumerate(widths):
            piece = asg[c][j]
            if piece is None:
                continue
            e, s0, n = piece
            tk = toks[e][s0:s0 + n]
            y[tk] += res.results[c][f"oe{j}"][:n]
            y[tk] += cw[tk, e][:, None] * b2[e][None, :]
        y[c * TS:(c + 1) * TS] += res.results[c]["zs"]
    y += bs2[None, :]
    return y.reshape(x.shape).astype(np.float32)
